# revision 3
# baseline (speedup 1.0000x reference)
"""Mixtral sparse MoE block on 8 Trainium2 NeuronCores (expert parallelism).

Strategy:
  - Host: router (gate matmul fp32 + softmax + top-2) and token dispatch.
    Each of the 8 experts is pinned to one core; tokens routed to expert e
    are gathered (with zero padding up to a uniform capacity C) and shipped
    to core e together with that expert's weights, pre-cast to bf16.
  - Device (SPMD, one program on 8 cores): the expert MLP
        y = (silu(x @ w1) * ((x * topw) @ w3)) @ w2
    computed as two PE phases. Phase A keeps f (FFN dim) on the PSUM
    partition axis so h is produced already transposed ([f, tokens]) and
    phase B's down-proj needs no on-chip transpose. The per-token routing
    weight is folded into the w3 operand on the host (u' = (x*s) @ w3), so
    the combine scaling costs nothing on device.
  - Host: scatter-add per-expert outputs back into the [T, H] result.

All matmuls run in bf16 with fp32 PSUM accumulation.
"""

import os
import sys

sys.path.insert(0, "/opt/trn_rl_repo")

from contextlib import ExitStack

import ml_dtypes
import numpy as np

import concourse.bass as bass  # noqa: F401  (engine types via nc)
import concourse.tile as tile
from concourse import bacc, mybir
from concourse.bass_utils import run_bass_kernel_spmd

H = 1024
F = 3584
E = 8
TOP_K = 2
P = 128
KT = H // P      # 8  k-tiles over hidden dim
FT = F // P      # 28 f-tiles over FFN dim
FB = F // 512    # 7  512-wide f-blocks for weight streaming

BF16 = ml_dtypes.bfloat16

_PROG_CACHE: dict[int, "bacc.Bacc"] = {}

# Overridable for CoreSim checks (CoreSim lacks a Silu implementation).
ACT = mybir.ActivationFunctionType.Silu

# Exposed for test harnesses: last BassKernelResults from the device run.
LAST_RESULTS = None


def _chunks(C, step=512):
    return [(c0, min(step, C - c0)) for c0 in range(0, C, step)]


def build_program(C: int) -> "bacc.Bacc":
    """Device program for capacity C (tokens per expert, multiple of 128)."""
    assert C % P == 0
    CT = C // P
    bf16, f32 = mybir.dt.bfloat16, mybir.dt.float32

    nc = bacc.Bacc("TRN2", target_bir_lowering=False, debug=False, num_devices=8)
    xT_d = nc.dram_tensor("xT", [H, C], bf16, kind="ExternalInput").ap()
    xsT_d = nc.dram_tensor("xsT", [H, C], bf16, kind="ExternalInput").ap()
    w1_d = nc.dram_tensor("w1", [H, F], bf16, kind="ExternalInput").ap()
    w3_d = nc.dram_tensor("w3", [H, F], bf16, kind="ExternalInput").ap()
    w2_d = nc.dram_tensor("w2", [F, H], bf16, kind="ExternalInput").ap()
    y_d = nc.dram_tensor("y", [C, H], f32, kind="ExternalOutput").ap()

    # [H, *] -> [p, k, *] views (p = partition within k-tile)
    xT_v = xT_d.rearrange("(k p) c -> p k c", p=P)
    xsT_v = xsT_d.rearrange("(k p) c -> p k c", p=P)
    w1_v = w1_d.rearrange("(k p) f -> p k f", p=P)
    w3_v = w3_d.rearrange("(k p) f -> p k f", p=P)

    with tile.TileContext(nc) as tc, ExitStack() as ctx:
        xpool = ctx.enter_context(tc.tile_pool(name="x", bufs=1))
        hpool = ctx.enter_context(tc.tile_pool(name="h", bufs=1))
        wpool = ctx.enter_context(tc.tile_pool(name="w", bufs=2))
        w2pool = ctx.enter_context(tc.tile_pool(name="w2", bufs=4))
        tpool = ctx.enter_context(tc.tile_pool(name="t", bufs=3))
        ypool = ctx.enter_context(tc.tile_pool(name="y", bufs=3))
        pspool = ctx.enter_context(tc.tile_pool(name="ps", bufs=4, space="PSUM"))

        xT_sb = xpool.tile([P, KT, C], bf16, tag="xT", name="xT_sb")
        nc.sync.dma_start(xT_sb[:], xT_v)
        xsT_sb = xpool.tile([P, KT, C], bf16, tag="xsT", name="xsT_sb")
        nc.sync.dma_start(xsT_sb[:], xsT_v)
        hT_sb = hpool.tile([P, FT, C], bf16, tag="hT", name="hT_sb")

        silu = ACT

        # ---- Phase A: hT[f, t] = silu(w1^T x) * (w3^T xs), f on partitions
        for fb in range(FB):
            w1b = wpool.tile([P, KT, 512], bf16, tag="w1b", name="w1b")
            nc.sync.dma_start(w1b[:], w1_v[:, :, fb * 512:(fb + 1) * 512])
            w3b = wpool.tile([P, KT, 512], bf16, tag="w3b", name="w3b")
            nc.sync.dma_start(w3b[:], w3_v[:, :, fb * 512:(fb + 1) * 512])
            for fi in range(4):
                ft = fb * 4 + fi
                for (c0, csz) in _chunks(C):
                    g_ps = pspool.tile([P, 512], f32, tag="ps", name="g_ps")[:, :csz]
                    u_ps = pspool.tile([P, 512], f32, tag="ps", name="u_ps")[:, :csz]
                    for k in range(KT):
                        nc.tensor.matmul(
                            g_ps,
                            lhsT=w1b[:, k, fi * P:(fi + 1) * P],
                            rhs=xT_sb[:, k, c0:c0 + csz],
                            start=(k == 0),
                            stop=(k == KT - 1),
                        )
                    for k in range(KT):
                        nc.tensor.matmul(
                            u_ps,
                            lhsT=w3b[:, k, fi * P:(fi + 1) * P],
                            rhs=xsT_sb[:, k, c0:c0 + csz],
                            start=(k == 0),
                            stop=(k == KT - 1),
                        )
                    sg = tpool.tile([P, 512], f32, tag="sg", name="sg")[:, :csz]
                    nc.scalar.activation(sg, g_ps, silu)
                    nc.vector.tensor_mul(hT_sb[:, ft, c0:c0 + csz], sg, u_ps)

        # ---- Phase B: y[t, h] = hT^T @ w2, streamed w2, token-tile groups
        GRP = 3
        for g0 in range(0, CT, GRP):
            tts = list(range(g0, min(g0 + GRP, CT)))
            yps = [
                pspool.tile([P, H], f32, tag="ps", name=f"y_ps{tt}") for tt in tts
            ]
            for f in range(FT):
                w2sb = w2pool.tile([P, H], bf16, tag="w2sb", name="w2sb")
                nc.sync.dma_start(w2sb[:], w2_d[f * P:(f + 1) * P, :])
                for i, tt in enumerate(tts):
                    for nh in range(0, H, 512):
                        nc.tensor.matmul(
                            yps[i][:, nh:nh + 512],
                            lhsT=hT_sb[:, f, tt * P:(tt + 1) * P],
                            rhs=w2sb[:, nh:nh + 512],
                            start=(f == 0),
                            stop=(f == FT - 1),
                        )
            for i, tt in enumerate(tts):
                ysb = ypool.tile([P, H], f32, tag="ysb", name="ysb")
                nc.scalar.copy(ysb[:], yps[i][:])
                nc.sync.dma_start(y_d[tt * P:(tt + 1) * P, :], ysb[:])

    nc.compile()
    return nc


def _route(x: np.ndarray, gate_w: np.ndarray):
    """fp32 router identical to the reference: softmax, top-2, renormalize."""
    logits = (x @ gate_w).astype(np.float32)  # [T, E]
    m = logits.max(axis=-1, keepdims=True)
    e = np.exp(logits - m)
    p = (e / e.sum(axis=-1, keepdims=True)).astype(np.float32)
    sel = np.argsort(-p, axis=-1, kind="stable")[:, :TOP_K]  # [T, k]
    tw = np.take_along_axis(p, sel, axis=-1)
    tw = (tw / tw.sum(axis=-1, keepdims=True)).astype(np.float32)
    return logits, sel, tw


def kernel(hidden_states, gate_w, w1, w2, w3):
    global LAST_RESULTS
    hidden_states = np.asarray(hidden_states, dtype=np.float32)
    gate_w = np.asarray(gate_w, dtype=np.float32)
    w1 = np.asarray(w1, dtype=np.float32)
    w2 = np.asarray(w2, dtype=np.float32)
    w3 = np.asarray(w3, dtype=np.float32)

    B, S, Hh = hidden_states.shape
    assert Hh == H
    x = hidden_states.reshape(-1, H)  # [T, H]
    T = x.shape[0]

    logits, sel, tw = _route(x, gate_w)

    # Per-expert token lists and routing weights
    idxs, wts = [], []
    for e in range(E):
        t_idx, k_idx = np.nonzero(sel == e)
        idxs.append(t_idx)
        wts.append(tw[t_idx, k_idx])
    counts = [len(i) for i in idxs]
    C = max(P, int(-(-max(counts) // P) * P))

    nc = _PROG_CACHE.get(C)
    if nc is None:
        nc = build_program(C)
        _PROG_CACHE[C] = nc

    in_maps = []
    for e in range(E):
        n = counts[e]
        xe = x[idxs[e]]                       # [n, H] fp32
        xT = np.zeros((H, C), dtype=BF16)
        xsT = np.zeros((H, C), dtype=BF16)
        xT[:, :n] = xe.T.astype(BF16)
        xsT[:, :n] = (xe * wts[e][:, None]).T.astype(BF16)
        in_maps.append(
            {
                "xT": xT,
                "xsT": xsT,
                "w1": np.ascontiguousarray(w1[e]).astype(BF16),
                "w3": np.ascontiguousarray(w3[e]).astype(BF16),
                "w2": np.ascontiguousarray(w2[e]).astype(BF16),
            }
        )

    res = run_bass_kernel_spmd(nc, in_maps, core_ids=list(range(8)))
    LAST_RESULTS = res

    final = np.zeros((T, H), dtype=np.float32)
    for e in range(E):
        n = counts[e]
        if n:
            final[idxs[e]] += res.results[e]["y"][:n]

    return final, logits


# revision 4
# speedup vs baseline: 1.0981x; 1.0981x over previous
"""Mixtral sparse MoE block on 8 Trainium2 NeuronCores (expert parallelism).

Strategy:
  - Host: router (gate matmul fp32 + softmax + top-2) and token dispatch.
    Each of the 8 experts is pinned to one core; tokens routed to expert e
    are gathered (zero-padded to a uniform capacity C) and shipped to core
    e together with that expert's weights, pre-cast to bf16.
  - Device (SPMD, one program on 8 cores): the expert MLP
        y = (silu(x @ w1) * ((x * topw) @ w3)) @ w2
    in two PE phases. Phase A keeps f (FFN dim) on the PSUM partition axis
    so h is produced already transposed ([f, tokens]) and phase B's
    down-proj needs no on-chip transpose. The per-token routing weight is
    folded into the w3 operand on the host (u' = (x*s) @ w3), so the
    combine scaling costs nothing on device.
  - Host: scatter-add per-expert outputs back into the [T, H] result.

All matmuls run in bf16 with fp32 PSUM accumulation.
"""

import os
import sys

sys.path.insert(0, "/opt/trn_rl_repo")

from contextlib import ExitStack

import ml_dtypes
import numpy as np

import concourse.bass as bass  # noqa: F401
import concourse.tile as tile
from concourse import bacc, mybir
from concourse.bass_utils import run_bass_kernel_spmd

H = 1024
F = 3584
E = 8
TOP_K = 2
P = 128
KT = H // P      # 8  k-tiles over hidden dim
FT = F // P      # 28 f-tiles over FFN dim
FB = F // 512    # 7  512-wide f-blocks for weight streaming

BF16 = ml_dtypes.bfloat16

_PROG_CACHE: dict[int, "bacc.Bacc"] = {}

# Overridable for CoreSim checks (CoreSim lacks a Silu implementation).
ACT = mybir.ActivationFunctionType.Silu

# Exposed for test harnesses: last BassKernelResults from the device run.
LAST_RESULTS = None


def _chunks(C, step=512):
    return [(c0, min(step, C - c0)) for c0 in range(0, C, step)]


def build_program(C: int) -> "bacc.Bacc":
    """Device program for capacity C tokens per expert (multiple of 16)."""
    assert C % 16 == 0
    CT = -(-C // P)  # token tiles, last may be partial
    tt_ranges = [(i * P, min(P, C - i * P)) for i in range(CT)]
    chunks = _chunks(C)
    bf16, f32 = mybir.dt.bfloat16, mybir.dt.float32

    nc = bacc.Bacc("TRN2", target_bir_lowering=False, debug=False, num_devices=8)
    xT_d = nc.dram_tensor("xT", [H, C], bf16, kind="ExternalInput").ap()
    xsT_d = nc.dram_tensor("xsT", [H, C], bf16, kind="ExternalInput").ap()
    w1_d = nc.dram_tensor("w1", [H, F], bf16, kind="ExternalInput").ap()
    w3_d = nc.dram_tensor("w3", [H, F], bf16, kind="ExternalInput").ap()
    w2_d = nc.dram_tensor("w2", [F, H], bf16, kind="ExternalInput").ap()
    y_d = nc.dram_tensor("y", [C, H], f32, kind="ExternalOutput").ap()

    # [H, *] -> [p, k, *] views (p = partition within k-tile)
    xT_v = xT_d.rearrange("(k p) c -> p k c", p=P)
    xsT_v = xsT_d.rearrange("(k p) c -> p k c", p=P)
    w1_v = w1_d.rearrange("(k p) f -> p k f", p=P)
    w3_v = w3_d.rearrange("(k p) f -> p k f", p=P)

    with tile.TileContext(nc) as tc, ExitStack() as ctx:
        xpool = ctx.enter_context(tc.tile_pool(name="x", bufs=1))
        hpool = ctx.enter_context(tc.tile_pool(name="h", bufs=1))
        wpool = ctx.enter_context(tc.tile_pool(name="w", bufs=2))
        w2pool = ctx.enter_context(tc.tile_pool(name="w2", bufs=8))
        tpool = ctx.enter_context(tc.tile_pool(name="t", bufs=6))
        ypool = ctx.enter_context(tc.tile_pool(name="y", bufs=3))
        warmpool = ctx.enter_context(tc.tile_pool(name="warm", bufs=1))

        # ---- PE warm-up: run junk matmuls on a zeroed tile while input DMAs
        # stream, so HAM reaches K=8/8 before the first real matmul.
        wz = warmpool.tile([P, 512], bf16, tag="wz", name="wz")
        nc.gpsimd.memset(wz[:], 0.0)

        # ---- Input DMAs, ordered for shortest time-to-first-real-matmul:
        # first weight block + first token chunk, then the rest.
        w1bs, w3bs = {}, {}
        w1bs[0] = wpool.tile([P, KT, 512], bf16, tag="w1b", name="w1b0")
        nc.sync.dma_start(w1bs[0][:], w1_v[:, :, 0:512])
        xT_sb = xpool.tile([P, KT, C], bf16, tag="xT", name="xT_sb")
        xsT_sb = xpool.tile([P, KT, C], bf16, tag="xsT", name="xsT_sb")
        (c0, csz0) = chunks[0]
        nc.sync.dma_start(xT_sb[:, :, c0:c0 + csz0], xT_v[:, :, c0:c0 + csz0])
        w3bs[0] = wpool.tile([P, KT, 512], bf16, tag="w3b", name="w3b0")
        nc.sync.dma_start(w3bs[0][:], w3_v[:, :, 0:512])
        nc.sync.dma_start(xsT_sb[:, :, c0:c0 + csz0], xsT_v[:, :, c0:c0 + csz0])
        for (c0, csz) in chunks[1:]:
            nc.sync.dma_start(xT_sb[:, :, c0:c0 + csz], xT_v[:, :, c0:c0 + csz])
            nc.sync.dma_start(xsT_sb[:, :, c0:c0 + csz], xsT_v[:, :, c0:c0 + csz])

        hT_sb = hpool.tile([P, FT, C], bf16, tag="hT", name="hT_sb")

        # ---- Phase A: hT[f, t] = silu(w1^T x) * (w3^T xs), f on partitions
        with tc.tile_pool(name="psA", bufs=8, space="PSUM") as psA:
            warm_ps = psA.tile([P, 512], f32, tag="ps", name="warm_ps")
            for i in range(16):
                nc.tensor.matmul(
                    warm_ps, lhsT=wz[:, :P], rhs=wz[:], start=True, stop=True
                )

            for fb in range(FB):
                if fb not in w1bs:
                    w1bs[fb] = wpool.tile([P, KT, 512], bf16, tag="w1b", name=f"w1b{fb}")
                    nc.sync.dma_start(w1bs[fb][:], w1_v[:, :, fb * 512:(fb + 1) * 512])
                    w3bs[fb] = wpool.tile([P, KT, 512], bf16, tag="w3b", name=f"w3b{fb}")
                    nc.sync.dma_start(w3bs[fb][:], w3_v[:, :, fb * 512:(fb + 1) * 512])
                w1b, w3b = w1bs[fb], w3bs[fb]
                for (c0, csz) in chunks:
                    g_list, sg_list = [], []
                    for fi in range(4):
                        g_ps = psA.tile([P, 512], f32, tag="ps", name="g_ps")[:, :csz]
                        for k in range(KT):
                            nc.tensor.matmul(
                                g_ps,
                                lhsT=w1b[:, k, fi * P:(fi + 1) * P],
                                rhs=xT_sb[:, k, c0:c0 + csz],
                                start=(k == 0),
                                stop=(k == KT - 1),
                            )
                        sg = tpool.tile([P, 512], f32, tag="sg", name="sg")[:, :csz]
                        nc.scalar.activation(sg, g_ps, ACT)
                        g_list.append(g_ps)
                        sg_list.append(sg)
                    for fi in range(4):
                        ft = fb * 4 + fi
                        u_ps = psA.tile([P, 512], f32, tag="ps", name="u_ps")[:, :csz]
                        for k in range(KT):
                            nc.tensor.matmul(
                                u_ps,
                                lhsT=w3b[:, k, fi * P:(fi + 1) * P],
                                rhs=xsT_sb[:, k, c0:c0 + csz],
                                start=(k == 0),
                                stop=(k == KT - 1),
                            )
                        nc.vector.tensor_mul(
                            hT_sb[:, ft, c0:c0 + csz], sg_list[fi], u_ps
                        )

        # ---- Phase B: y[t, h] = hT^T @ w2, streamed w2, token-tile groups
        GRP = 3
        with tc.tile_pool(name="psB", bufs=4, space="PSUM") as psB:
            for g0 in range(0, CT, GRP):
                tts = list(range(g0, min(g0 + GRP, CT)))
                yps = [psB.tile([P, H], f32, tag="ps", name=f"y_ps{tt}") for tt in tts]
                for f in range(FT):
                    w2sb = w2pool.tile([P, H], bf16, tag="w2sb", name="w2sb")
                    nc.sync.dma_start(w2sb[:], w2_d[f * P:(f + 1) * P, :])
                    for i, tt in enumerate(tts):
                        s, sz = tt_ranges[tt]
                        for nh in range(0, H, 512):
                            nc.tensor.matmul(
                                yps[i][:sz, nh:nh + 512],
                                lhsT=hT_sb[:, f, s:s + sz],
                                rhs=w2sb[:, nh:nh + 512],
                                start=(f == 0),
                                stop=(f == FT - 1),
                            )
                for i, tt in enumerate(tts):
                    s, sz = tt_ranges[tt]
                    ysb = ypool.tile([P, H], f32, tag="ysb", name="ysb")
                    nc.scalar.copy(ysb[:sz, :], yps[i][:sz, :])
                    nc.sync.dma_start(y_d[s:s + sz, :], ysb[:sz, :])

    nc.compile()
    return nc


def _route(x: np.ndarray, gate_w: np.ndarray):
    """fp32 router identical to the reference: softmax, top-2, renormalize."""
    logits = (x @ gate_w).astype(np.float32)  # [T, E]
    m = logits.max(axis=-1, keepdims=True)
    e = np.exp(logits - m)
    p = (e / e.sum(axis=-1, keepdims=True)).astype(np.float32)
    sel = np.argsort(-p, axis=-1, kind="stable")[:, :TOP_K]  # [T, k]
    tw = np.take_along_axis(p, sel, axis=-1)
    tw = (tw / tw.sum(axis=-1, keepdims=True)).astype(np.float32)
    return logits, sel, tw


def kernel(hidden_states, gate_w, w1, w2, w3):
    global LAST_RESULTS
    hidden_states = np.asarray(hidden_states, dtype=np.float32)
    gate_w = np.asarray(gate_w, dtype=np.float32)
    w1 = np.asarray(w1, dtype=np.float32)
    w2 = np.asarray(w2, dtype=np.float32)
    w3 = np.asarray(w3, dtype=np.float32)

    B, S, Hh = hidden_states.shape
    assert Hh == H
    x = hidden_states.reshape(-1, H)  # [T, H]
    T = x.shape[0]

    logits, sel, tw = _route(x, gate_w)

    # Per-expert token lists and routing weights
    idxs, wts = [], []
    for e in range(E):
        t_idx, k_idx = np.nonzero(sel == e)
        idxs.append(t_idx)
        wts.append(tw[t_idx, k_idx])
    counts = [len(i) for i in idxs]
    C = max(P, int(-(-max(counts) // 16) * 16))

    nc = _PROG_CACHE.get(C)
    if nc is None:
        nc = build_program(C)
        _PROG_CACHE[C] = nc

    in_maps = []
    for e in range(E):
        n = counts[e]
        xe = x[idxs[e]]                       # [n, H] fp32
        xT = np.zeros((H, C), dtype=BF16)
        xsT = np.zeros((H, C), dtype=BF16)
        xT[:, :n] = xe.T.astype(BF16)
        xsT[:, :n] = (xe * wts[e][:, None]).T.astype(BF16)
        in_maps.append(
            {
                "xT": xT,
                "xsT": xsT,
                "w1": np.ascontiguousarray(w1[e]).astype(BF16),
                "w3": np.ascontiguousarray(w3[e]).astype(BF16),
                "w2": np.ascontiguousarray(w2[e]).astype(BF16),
            }
        )

    res = run_bass_kernel_spmd(nc, in_maps, core_ids=list(range(8)))
    LAST_RESULTS = res

    final = np.zeros((T, H), dtype=np.float32)
    for e in range(E):
        n = counts[e]
        if n:
            final[idxs[e]] += res.results[e]["y"][:n]

    return final, logits


# revision 11
# speedup vs baseline: 1.1197x; 1.0196x over previous
"""Mixtral sparse MoE block on 8 Trainium2 NeuronCores (expert parallelism).

Strategy:
  - Host: router (gate matmul fp32 + softmax + top-2) and token dispatch.
    Each of the 8 experts is pinned to one core; tokens routed to expert e
    are gathered (zero-padded to a uniform capacity C) and shipped to core
    e together with that expert's weights, pre-cast to bf16.
  - Device (SPMD, one program on 8 cores): the expert MLP
        y = (silu(x @ w1) * ((x * topw) @ w3)) @ w2
    in two PE phases. Phase A keeps f (FFN dim) on the PSUM partition axis
    so h is produced already transposed ([f, tokens]) and phase B's
    down-proj needs no on-chip transpose. The per-token routing weight is
    folded into the w3 operand on the host (u' = (x*s) @ w3), so the
    combine scaling costs nothing on device.
  - Host: scatter-add per-expert outputs back into the [T, H] result.

All matmuls run in bf16 with fp32 PSUM accumulation.
"""

import os
import sys

sys.path.insert(0, "/opt/trn_rl_repo")

from contextlib import ExitStack

import ml_dtypes
import numpy as np

import concourse.bass as bass  # noqa: F401
import concourse.tile as tile
from concourse import bacc, mybir
from concourse.bass_utils import run_bass_kernel_spmd

H = 1024
F = 3584
E = 8
TOP_K = 2
P = 128
KT = H // P      # 8  k-tiles over hidden dim
FT = F // P      # 28 f-tiles over FFN dim
FB = F // 512    # 7  512-wide f-blocks for weight streaming

BF16 = ml_dtypes.bfloat16

_PROG_CACHE: dict[int, "bacc.Bacc"] = {}

# Overridable for CoreSim checks (CoreSim lacks a Silu implementation).
ACT = mybir.ActivationFunctionType.Silu

# Exposed for test harnesses: last BassKernelResults from the device run.
LAST_RESULTS = None


def _chunks(C, step=512):
    """Split C into near-equal 16-aligned chunks of at most `step`."""
    n = -(-C // step)
    base = (C // n) // 16 * 16
    sizes = [base] * n
    extra = C - base * n
    i = 0
    while extra >= 16:
        sizes[i] += 16
        extra -= 16
        i = (i + 1) % n
    if extra:
        sizes[-1] += extra
    out, c0 = [], 0
    for s in sizes:
        out.append((c0, s))
        c0 += s
    return out


def build_program(C: int) -> "bacc.Bacc":
    """Device program for capacity C tokens per expert (multiple of 16)."""
    assert C % 16 == 0
    chunks = _chunks(C)
    bf16, f32 = mybir.dt.bfloat16, mybir.dt.float32

    nc = bacc.Bacc("TRN2", target_bir_lowering=False, debug=False, num_devices=8)
    xT_d = nc.dram_tensor("xT", [H, C], bf16, kind="ExternalInput").ap()
    xsT_d = nc.dram_tensor("xsT", [H, C], bf16, kind="ExternalInput").ap()
    w1_d = nc.dram_tensor("w1", [H, F], bf16, kind="ExternalInput").ap()
    w3_d = nc.dram_tensor("w3", [H, F], bf16, kind="ExternalInput").ap()
    w2_d = nc.dram_tensor("w2", [F, H], bf16, kind="ExternalInput").ap()
    yT_d = nc.dram_tensor("yT", [H, C], f32, kind="ExternalOutput").ap()

    # [H, *] -> [p, k, *] views (p = partition within k-tile)
    xT_v = xT_d.rearrange("(k p) c -> p k c", p=P)
    xsT_v = xsT_d.rearrange("(k p) c -> p k c", p=P)
    w1_v = w1_d.rearrange("(k p) f -> p k f", p=P)
    w3_v = w3_d.rearrange("(k p) f -> p k f", p=P)

    with tile.TileContext(nc) as tc, ExitStack() as ctx:
        xpool = ctx.enter_context(tc.tile_pool(name="x", bufs=1))
        hpool = ctx.enter_context(tc.tile_pool(name="h", bufs=1))
        wpool = ctx.enter_context(tc.tile_pool(name="w", bufs=2))
        w2pool = ctx.enter_context(tc.tile_pool(name="w2", bufs=8))
        tpool = ctx.enter_context(tc.tile_pool(name="t", bufs=6))
        ypool = ctx.enter_context(tc.tile_pool(name="y", bufs=4))
        warmpool = ctx.enter_context(tc.tile_pool(name="warm", bufs=1))

        # ---- PE warm-up: run junk matmuls on a zeroed tile while input DMAs
        # stream, so HAM reaches K=8/8 before the first real matmul.
        wz = warmpool.tile([P, 512], bf16, tag="wz", name="wz")
        nc.gpsimd.memset(wz[:], 0.0)
        N_WARM = 22

        # ---- Input DMAs, ordered for shortest time-to-first-real-matmul:
        # first weight block + first token chunk, then the rest.
        w1bs, w3bs = {}, {}
        w1bs[0] = wpool.tile([P, KT, 512], bf16, tag="w1b", name="w1b0")
        nc.sync.dma_start(w1bs[0][:], w1_v[:, :, 0:512])
        xT_sb = xpool.tile([P, KT, C], bf16, tag="xT", name="xT_sb")
        xsT_sb = xpool.tile([P, KT, C], bf16, tag="xsT", name="xsT_sb")
        (c0, csz0) = chunks[0]
        nc.sync.dma_start(xT_sb[:, :, c0:c0 + csz0], xT_v[:, :, c0:c0 + csz0])
        w3bs[0] = wpool.tile([P, KT, 512], bf16, tag="w3b", name="w3b0")
        nc.sync.dma_start(w3bs[0][:], w3_v[:, :, 0:512])
        nc.sync.dma_start(xsT_sb[:, :, c0:c0 + csz0], xsT_v[:, :, c0:c0 + csz0])
        for (c0, csz) in chunks[1:]:
            nc.sync.dma_start(xT_sb[:, :, c0:c0 + csz], xT_v[:, :, c0:c0 + csz])
            nc.sync.dma_start(xsT_sb[:, :, c0:c0 + csz], xsT_v[:, :, c0:c0 + csz])

        hT_sb = hpool.tile([P, FT, C], bf16, tag="hT", name="hT_sb")

        # Single PSUM pool shared by both phases: 8 one-bank slots.
        ps = ctx.enter_context(tc.tile_pool(name="ps", bufs=8, space="PSUM"))

        warm_ps = ps.tile([P, 512], f32, tag="ps", name="warm_ps")
        for i in range(N_WARM):
            nc.tensor.matmul(
                warm_ps, lhsT=wz[:, :P], rhs=wz[:], start=True, stop=True
            )

        # ---- Phase A: hT[f, t] = silu(w1^T x) * (w3^T xs), f on partitions
        for fb in range(FB):
            if fb not in w1bs:
                w1bs[fb] = wpool.tile([P, KT, 512], bf16, tag="w1b", name=f"w1b{fb}")
                nc.sync.dma_start(w1bs[fb][:], w1_v[:, :, fb * 512:(fb + 1) * 512])
                w3bs[fb] = wpool.tile([P, KT, 512], bf16, tag="w3b", name=f"w3b{fb}")
                nc.sync.dma_start(w3bs[fb][:], w3_v[:, :, fb * 512:(fb + 1) * 512])
            w1b, w3b = w1bs[fb], w3bs[fb]
            for (c0, csz) in chunks:
                sg_list = []
                for fi in range(4):
                    g_ps = ps.tile([P, 512], f32, tag="ps", name="g_ps")[:, :csz]
                    for k in range(KT):
                        nc.tensor.matmul(
                            g_ps,
                            lhsT=w1b[:, k, fi * P:(fi + 1) * P],
                            rhs=xT_sb[:, k, c0:c0 + csz],
                            start=(k == 0),
                            stop=(k == KT - 1),
                        )
                    sg = tpool.tile([P, 512], f32, tag="sg", name="sg")[:, :csz]
                    nc.scalar.activation(sg, g_ps, ACT)
                    sg_list.append(sg)
                for fi in range(4):
                    ft = fb * 4 + fi
                    u_ps = ps.tile([P, 512], f32, tag="ps", name="u_ps")[:, :csz]
                    for k in range(KT):
                        nc.tensor.matmul(
                            u_ps,
                            lhsT=w3b[:, k, fi * P:(fi + 1) * P],
                            rhs=xsT_sb[:, k, c0:c0 + csz],
                            start=(k == 0),
                            stop=(k == KT - 1),
                        )
                    nc.vector.tensor_mul(
                        hT_sb[:, ft, c0:c0 + csz], sg_list[fi], u_ps
                    )

        # ---- Phase B: yT[h, t] = w2^T @ h.  w2 f-slabs stream (once per
        # token chunk); hT is the moving operand so PE cost scales with C.
        # All 8 h-tiles accumulate in PSUM simultaneously (8 banks).
        for (c0, csz) in chunks:
            yps = [ps.tile([P, 512], f32, tag="ps", name=f"yT_ps{ht}")[:, :csz]
                   for ht in range(KT)]
            for f in range(FT):
                w2sb = w2pool.tile([P, H], bf16, tag="w2sb", name="w2sb")
                nc.sync.dma_start(w2sb[:], w2_d[f * P:(f + 1) * P, :])
                for ht in range(KT):
                    nc.tensor.matmul(
                        yps[ht],
                        lhsT=w2sb[:, ht * P:(ht + 1) * P],
                        rhs=hT_sb[:, f, c0:c0 + csz],
                        start=(f == 0),
                        stop=(f == FT - 1),
                    )
            for ht in range(KT):
                ysb = ypool.tile([P, 512], f32, tag="ysb", name="ysb")[:, :csz]
                if ht % 2 == 0:
                    nc.scalar.copy(ysb, yps[ht])
                else:
                    nc.vector.tensor_copy(ysb, yps[ht])
                nc.sync.dma_start(yT_d[ht * P:(ht + 1) * P, c0:c0 + csz], ysb)

    nc.compile()
    return nc


def _route(x: np.ndarray, gate_w: np.ndarray):
    """fp32 router identical to the reference: softmax, top-2, renormalize."""
    logits = (x @ gate_w).astype(np.float32)  # [T, E]
    m = logits.max(axis=-1, keepdims=True)
    e = np.exp(logits - m)
    p = (e / e.sum(axis=-1, keepdims=True)).astype(np.float32)
    sel = np.argsort(-p, axis=-1, kind="stable")[:, :TOP_K]  # [T, k]
    tw = np.take_along_axis(p, sel, axis=-1)
    tw = (tw / tw.sum(axis=-1, keepdims=True)).astype(np.float32)
    return logits, sel, tw


def kernel(hidden_states, gate_w, w1, w2, w3):
    global LAST_RESULTS
    hidden_states = np.asarray(hidden_states, dtype=np.float32)
    gate_w = np.asarray(gate_w, dtype=np.float32)
    w1 = np.asarray(w1, dtype=np.float32)
    w2 = np.asarray(w2, dtype=np.float32)
    w3 = np.asarray(w3, dtype=np.float32)

    B, S, Hh = hidden_states.shape
    assert Hh == H
    x = hidden_states.reshape(-1, H)  # [T, H]
    T = x.shape[0]

    logits, sel, tw = _route(x, gate_w)

    # Per-expert token lists and routing weights
    idxs, wts = [], []
    for e in range(E):
        t_idx, k_idx = np.nonzero(sel == e)
        idxs.append(t_idx)
        wts.append(tw[t_idx, k_idx])
    counts = [len(i) for i in idxs]
    C = max(P, int(-(-max(counts) // 16) * 16))

    nc = _PROG_CACHE.get(C)
    if nc is None:
        nc = build_program(C)
        _PROG_CACHE[C] = nc

    in_maps = []
    for e in range(E):
        n = counts[e]
        xe = x[idxs[e]]                       # [n, H] fp32
        xT = np.zeros((H, C), dtype=BF16)
        xsT = np.zeros((H, C), dtype=BF16)
        xT[:, :n] = xe.T.astype(BF16)
        xsT[:, :n] = (xe * wts[e][:, None]).T.astype(BF16)
        in_maps.append(
            {
                "xT": xT,
                "xsT": xsT,
                "w1": np.ascontiguousarray(w1[e]).astype(BF16),
                "w3": np.ascontiguousarray(w3[e]).astype(BF16),
                "w2": np.ascontiguousarray(w2[e]).astype(BF16),
            }
        )

    res = run_bass_kernel_spmd(nc, in_maps, core_ids=list(range(8)))
    LAST_RESULTS = res

    final = np.zeros((T, H), dtype=np.float32)
    for e in range(E):
        n = counts[e]
        if n:
            final[idxs[e]] += res.results[e]["yT"][:, :n].T

    return final, logits


# revision 15
# speedup vs baseline: 1.1213x; 1.0014x over previous
"""Mixtral sparse MoE block on 8 Trainium2 NeuronCores (expert parallelism).

Strategy:
  - Host: router (gate matmul fp32 + softmax + top-2) and token dispatch.
    Each of the 8 experts is pinned to one core; tokens routed to expert e
    are gathered (zero-padded to a uniform capacity C) and shipped to core
    e together with that expert's weights, pre-cast to bf16.
  - Device (SPMD, one program on 8 cores): the expert MLP
        y = (silu(x @ w1) * ((x * topw) @ w3)) @ w2
    in two PE phases. Phase A keeps f (FFN dim) on the PSUM partition axis
    so h is produced already transposed ([f, tokens]) and phase B's
    down-proj needs no on-chip transpose. The per-token routing weight is
    folded into the w3 operand on the host (u' = (x*s) @ w3), so the
    combine scaling costs nothing on device.
  - Host: scatter-add per-expert outputs back into the [T, H] result.

All matmuls run in bf16 with fp32 PSUM accumulation.
"""

import os
import sys

sys.path.insert(0, "/opt/trn_rl_repo")

from contextlib import ExitStack

import ml_dtypes
import numpy as np

import concourse.bass as bass  # noqa: F401
import concourse.tile as tile
from concourse import bacc, mybir
from concourse.bass_utils import run_bass_kernel_spmd

H = 1024
F = 3584
E = 8
TOP_K = 2
P = 128
KT = H // P      # 8  k-tiles over hidden dim
FT = F // P      # 28 f-tiles over FFN dim
FB = F // 512    # 7  512-wide f-blocks for weight streaming

BF16 = ml_dtypes.bfloat16

_PROG_CACHE: dict[int, "bacc.Bacc"] = {}

# Overridable for CoreSim checks (CoreSim lacks a Silu implementation).
ACT = mybir.ActivationFunctionType.Silu

# Exposed for test harnesses: last BassKernelResults from the device run.
LAST_RESULTS = None


def _chunks(C, step=512):
    """Split C into 16-aligned chunks <= step, each >= 256 (so LDWEIGHTS
    hides behind the matmul), with a small 256 chunk last to shorten the
    final PSUM evacuation."""
    if C <= step:
        return [(0, C)]
    R = C - 256
    n = -(-R // step)
    base = (R // n) // 16 * 16
    sizes = [base] * n
    extra = R - base * n
    i = 0
    while extra >= 16:
        sizes[i] += 16
        extra -= 16
        i = (i + 1) % n
    if extra:
        sizes[-1] += extra
    sizes.append(256)
    out, c0 = [], 0
    for s in sizes:
        out.append((c0, s))
        c0 += s
    return out


def build_program(C: int) -> "bacc.Bacc":
    """Device program for capacity C tokens per expert (multiple of 16)."""
    assert C % 16 == 0
    chunks = _chunks(C)
    bf16, f32 = mybir.dt.bfloat16, mybir.dt.float32

    nc = bacc.Bacc("TRN2", target_bir_lowering=False, debug=False, num_devices=8)
    xT_d = nc.dram_tensor("xT", [H, C], bf16, kind="ExternalInput").ap()
    xsT_d = nc.dram_tensor("xsT", [H, C], bf16, kind="ExternalInput").ap()
    w1_d = nc.dram_tensor("w1", [H, F], bf16, kind="ExternalInput").ap()
    w3_d = nc.dram_tensor("w3", [H, F], bf16, kind="ExternalInput").ap()
    w2_d = nc.dram_tensor("w2", [F, H], bf16, kind="ExternalInput").ap()
    yT_d = nc.dram_tensor("yT", [H, C], f32, kind="ExternalOutput").ap()

    # [H, *] -> [p, k, *] views (p = partition within k-tile)
    xT_v = xT_d.rearrange("(k p) c -> p k c", p=P)
    xsT_v = xsT_d.rearrange("(k p) c -> p k c", p=P)
    w1_v = w1_d.rearrange("(k p) f -> p k f", p=P)
    w3_v = w3_d.rearrange("(k p) f -> p k f", p=P)

    with tile.TileContext(nc) as tc, ExitStack() as ctx:
        xpool = ctx.enter_context(tc.tile_pool(name="x", bufs=1))
        hpool = ctx.enter_context(tc.tile_pool(name="h", bufs=1))
        wpool = ctx.enter_context(tc.tile_pool(name="w", bufs=2))
        w2pool = ctx.enter_context(tc.tile_pool(name="w2", bufs=8))
        tpool = ctx.enter_context(tc.tile_pool(name="t", bufs=6))
        ypool = ctx.enter_context(tc.tile_pool(name="y", bufs=4))
        warmpool = ctx.enter_context(tc.tile_pool(name="warm", bufs=1))

        # ---- PE warm-up: run junk matmuls on a zeroed tile while input DMAs
        # stream, so HAM reaches K=8/8 before the first real matmul.
        wz = warmpool.tile([P, 512], bf16, tag="wz", name="wz")
        nc.gpsimd.memset(wz[:], 0.0)
        N_WARM = 12

        # ---- Input DMAs, ordered for shortest time-to-first-real-matmul:
        # first weight block + first token chunk, then the rest.
        # First f-tile of w1 and first half of the first x chunk land first,
        # so the first real matmul can issue ~4us earlier.
        w1bs, w3bs = {}, {}
        w1bs[0] = wpool.tile([P, KT, 512], bf16, tag="w1b", name="w1b0")
        nc.sync.dma_start(w1bs[0][:, :, 0:P], w1_v[:, :, 0:P])
        xT_sb = xpool.tile([P, KT, C], bf16, tag="xT", name="xT_sb")
        xsT_sb = xpool.tile([P, KT, C], bf16, tag="xsT", name="xsT_sb")
        (c0, csz0) = chunks[0]
        nc.sync.dma_start(xT_sb[:, 0:4, c0:c0 + csz0], xT_v[:, 0:4, c0:c0 + csz0])
        nc.sync.dma_start(xT_sb[:, 4:8, c0:c0 + csz0], xT_v[:, 4:8, c0:c0 + csz0])
        nc.sync.dma_start(w1bs[0][:, :, P:512], w1_v[:, :, P:512])
        w3bs[0] = wpool.tile([P, KT, 512], bf16, tag="w3b", name="w3b0")
        nc.sync.dma_start(w3bs[0][:], w3_v[:, :, 0:512])
        nc.sync.dma_start(xsT_sb[:, :, c0:c0 + csz0], xsT_v[:, :, c0:c0 + csz0])
        for (c0, csz) in chunks[1:]:
            nc.sync.dma_start(xT_sb[:, :, c0:c0 + csz], xT_v[:, :, c0:c0 + csz])
            nc.sync.dma_start(xsT_sb[:, :, c0:c0 + csz], xsT_v[:, :, c0:c0 + csz])

        hT_sb = hpool.tile([P, FT, C], bf16, tag="hT", name="hT_sb")

        # Single PSUM pool shared by both phases: 8 one-bank slots.
        ps = ctx.enter_context(tc.tile_pool(name="ps", bufs=8, space="PSUM"))

        warm_ps = ps.tile([P, 512], f32, tag="ps", name="warm_ps")
        for i in range(N_WARM):
            nc.tensor.matmul(
                warm_ps, lhsT=wz[:, :P], rhs=wz[:], start=True, stop=True
            )

        # ---- Phase A: hT[f, t] = silu(w1^T x) * (w3^T xs), f on partitions
        for fb in range(FB):
            if fb not in w1bs:
                w1bs[fb] = wpool.tile([P, KT, 512], bf16, tag="w1b", name=f"w1b{fb}")
                nc.sync.dma_start(w1bs[fb][:], w1_v[:, :, fb * 512:(fb + 1) * 512])
                w3bs[fb] = wpool.tile([P, KT, 512], bf16, tag="w3b", name=f"w3b{fb}")
                nc.sync.dma_start(w3bs[fb][:], w3_v[:, :, fb * 512:(fb + 1) * 512])
            w1b, w3b = w1bs[fb], w3bs[fb]
            for (c0, csz) in chunks:
                sg_list = []
                for fi in range(4):
                    g_ps = ps.tile([P, 512], f32, tag="ps", name="g_ps")[:, :csz]
                    for k in range(KT):
                        nc.tensor.matmul(
                            g_ps,
                            lhsT=w1b[:, k, fi * P:(fi + 1) * P],
                            rhs=xT_sb[:, k, c0:c0 + csz],
                            start=(k == 0),
                            stop=(k == KT - 1),
                        )
                    sg = tpool.tile([P, 512], f32, tag="sg", name="sg")[:, :csz]
                    nc.scalar.activation(sg, g_ps, ACT)
                    sg_list.append(sg)
                for fi in range(4):
                    ft = fb * 4 + fi
                    u_ps = ps.tile([P, 512], f32, tag="ps", name="u_ps")[:, :csz]
                    for k in range(KT):
                        nc.tensor.matmul(
                            u_ps,
                            lhsT=w3b[:, k, fi * P:(fi + 1) * P],
                            rhs=xsT_sb[:, k, c0:c0 + csz],
                            start=(k == 0),
                            stop=(k == KT - 1),
                        )
                    nc.vector.tensor_mul(
                        hT_sb[:, ft, c0:c0 + csz], sg_list[fi], u_ps
                    )

        # ---- Phase B: yT[h, t] = w2^T @ h.  w2 f-slabs stream (once per
        # token chunk); hT is the moving operand so PE cost scales with C.
        # All 8 h-tiles accumulate in PSUM simultaneously (8 banks).
        w2p_v = w2_d.rearrange("(fp p) h -> p fp h", p=P)  # [128, 28, 1024]
        for ci, (c0, csz) in enumerate(chunks):
            last_chunk = ci == len(chunks) - 1
            yps = [ps.tile([P, 512], f32, tag="ps", name=f"yT_ps{ht}")[:, :csz]
                   for ht in range(KT)]
            for f2 in range(FT // 2):
                w2sb = w2pool.tile([P, 2, H], bf16, tag="w2sb", name="w2sb")
                nc.sync.dma_start(w2sb[:], w2p_v[:, 2 * f2:2 * f2 + 2, :])
                for j in range(2):
                    f = 2 * f2 + j
                    for ht in range(KT):
                        nc.tensor.matmul(
                            yps[ht],
                            lhsT=w2sb[:, j, ht * P:(ht + 1) * P],
                            rhs=hT_sb[:, f, c0:c0 + csz],
                            start=(f == 0),
                            stop=(f == FT - 1),
                        )
            for ht in range(KT):
                ysb = ypool.tile([P, 512], f32, tag="ysb", name="ysb")[:, :csz]
                if ht % 2 == 0:
                    nc.scalar.copy(ysb, yps[ht])
                else:
                    nc.vector.tensor_copy(ysb, yps[ht])
                # On the final chunk, split output-DMA issue across two
                # engines so descriptor generation doesn't serialize the tail.
                dma_eng = nc.gpsimd if (last_chunk and ht % 2 == 1) else nc.sync
                dma_eng.dma_start(yT_d[ht * P:(ht + 1) * P, c0:c0 + csz], ysb)

    nc.compile()
    return nc


def _route(x: np.ndarray, gate_w: np.ndarray):
    """fp32 router identical to the reference: softmax, top-2, renormalize."""
    logits = (x @ gate_w).astype(np.float32)  # [T, E]
    m = logits.max(axis=-1, keepdims=True)
    e = np.exp(logits - m)
    p = (e / e.sum(axis=-1, keepdims=True)).astype(np.float32)
    sel = np.argsort(-p, axis=-1, kind="stable")[:, :TOP_K]  # [T, k]
    tw = np.take_along_axis(p, sel, axis=-1)
    tw = (tw / tw.sum(axis=-1, keepdims=True)).astype(np.float32)
    return logits, sel, tw


def kernel(hidden_states, gate_w, w1, w2, w3):
    global LAST_RESULTS
    hidden_states = np.asarray(hidden_states, dtype=np.float32)
    gate_w = np.asarray(gate_w, dtype=np.float32)
    w1 = np.asarray(w1, dtype=np.float32)
    w2 = np.asarray(w2, dtype=np.float32)
    w3 = np.asarray(w3, dtype=np.float32)

    B, S, Hh = hidden_states.shape
    assert Hh == H
    x = hidden_states.reshape(-1, H)  # [T, H]
    T = x.shape[0]

    logits, sel, tw = _route(x, gate_w)

    # Per-expert token lists and routing weights
    idxs, wts = [], []
    for e in range(E):
        t_idx, k_idx = np.nonzero(sel == e)
        idxs.append(t_idx)
        wts.append(tw[t_idx, k_idx])
    counts = [len(i) for i in idxs]
    C = max(P, int(-(-max(counts) // 16) * 16))

    nc = _PROG_CACHE.get(C)
    if nc is None:
        nc = build_program(C)
        _PROG_CACHE[C] = nc

    in_maps = []
    for e in range(E):
        n = counts[e]
        xe = x[idxs[e]]                       # [n, H] fp32
        xT = np.zeros((H, C), dtype=BF16)
        xsT = np.zeros((H, C), dtype=BF16)
        xT[:, :n] = xe.T.astype(BF16)
        xsT[:, :n] = (xe * wts[e][:, None]).T.astype(BF16)
        in_maps.append(
            {
                "xT": xT,
                "xsT": xsT,
                "w1": np.ascontiguousarray(w1[e]).astype(BF16),
                "w3": np.ascontiguousarray(w3[e]).astype(BF16),
                "w2": np.ascontiguousarray(w2[e]).astype(BF16),
            }
        )

    res = run_bass_kernel_spmd(nc, in_maps, core_ids=list(range(8)))
    LAST_RESULTS = res

    final = np.zeros((T, H), dtype=np.float32)
    for e in range(E):
        n = counts[e]
        if n:
            final[idxs[e]] += res.results[e]["yT"][:, :n].T

    return final, logits
